# revision 6
# baseline (speedup 1.0000x reference)
"""BinarizeLinear Trainium2 kernel.

Computes y = binarize(x) @ binarize(W)^T + bias where binarize(t) = where(t>0, +1, -1),
x: [8192, 4096] f32, W: [4096, 4096] f32, bias: [4096] f32.

Strategy (8 NeuronCores, data parallel over tokens):
  - Each core gets 1024 tokens: xT shard [D_IN, 1024] (host-transposed layout so the
    contraction dim lands on SBUF partitions), the full W^T [D_IN, D_OUT], and bias.
  - On device: binarize via ScalarE Sign(t - 1e-20) (exact where(t>0,+1,-1) semantics,
    incl. t==0 -> -1) into fp8e4m3 (+/-1 exact in fp8; accumulation is fp32 PSUM so the
    whole matmul is exact). x kept resident in SBUF as fp8; W streamed fp32 -> fp8.
  - TensorE matmul in DoubleRow fp8 mode (2 MACs/cell/cycle), out^T[o, m] tiles in PSUM,
    bias added during the PSUM->SBUF drain on VectorE (bias is per-partition in this
    orientation), then DMA to DRAM as outT [D_OUT, 1024].
  - Host gathers the 8 outT shards and transposes back to [8192, 4096].
"""

import os
import sys

import numpy as np

sys.path.insert(0, "/opt/trn_rl_repo")

import concourse.bacc as bacc
import concourse.mybir as mybir
import concourse.tile as tile
from concourse.bass import ts
from concourse.bass_utils import run_bass_kernel_spmd
from concourse.kernels.tile_matmul import (
    ShapeInfo,
    composable_matmul_tile_kernel,
    dma_to_dram_mxn,
)

N_TOK, D_IN, D_OUT = 8192, 4096, 4096
NCORES = 8
M_LOC = N_TOK // NCORES  # tokens per core
P = 128
KT = 4  # K subtiles per K tile (K_TILE = 512)
NEG_TINY = -1.0e-20  # Sign(t + NEG_TINY): t>0 -> +1, t<=0 -> -1

F32 = mybir.dt.float32
FP8 = mybir.dt.float8e4
SIGN = mybir.ActivationFunctionType.Sign

LAST_EXEC_TIME_NS = None


def build(d_in=D_IN, d_out=D_OUT, m_loc=M_LOC):
    """Build the per-core Bass program (SPMD: all cores run the same NEFF)."""
    k_tiles = d_in // (P * KT)
    nc = bacc.Bacc("TRN2", target_bir_lowering=False, debug=False)

    xT = nc.dram_tensor("xT", (d_in, m_loc), F32, kind="ExternalInput")
    wT = nc.dram_tensor("wT", (d_in, d_out), F32, kind="ExternalInput")
    bias = nc.dram_tensor("bias", (d_out,), F32, kind="ExternalInput")
    outT = nc.dram_tensor("outT", (d_out, m_loc), F32, kind="ExternalOutput")

    with tile.TileContext(nc) as tc:
        with (
            tc.tile_pool(name="const", bufs=1) as const,
            tc.tile_pool(name="xstage", bufs=2) as xstage,
            tc.tile_pool(name="xb", bufs=1) as xbpool,
            tc.tile_pool(name="wstage", bufs=4) as wstage,
            tc.tile_pool(name="kxm", bufs=k_tiles + 1) as kxm_pool,
        ):
            bias_sb = const.tile([P, d_out // P], F32)
            nc.sync.dma_start(bias_sb[:], bias[:].rearrange("(o p) -> p o", p=P))

            # Per-partition bias for Sign(t + NEG_TINY) (float biases need a const AP)
            sign_bias = const.tile([P, 1], F32)
            nc.any.memset(sign_bias[:], NEG_TINY)

            xT_t = xT[:].rearrange("(po pi) m -> pi po m", pi=P)  # [128, d_in/128, m]
            wT_t = wT[:].rearrange("(po pi) o -> pi po o", pi=P)  # [128, d_in/128, o]

            # Load + binarize x once; keep resident in SBUF as fp8 (one tile per K_TILE
            # so matmul dependencies are per-K-tile, letting PE start before the full
            # load finishes). x DMAs go on the scalar HWDGE ring so the long W stream
            # on the sync ring doesn't queue behind them (and vice versa).
            xb_tiles = []
            for kt in range(k_tiles):
                stg = xstage.tile([P, KT, m_loc], F32, tag="xstage")
                nc.scalar.dma_start(stg[:], xT_t[:, ts(kt, KT), :])
                xb = xbpool.tile([P, KT, m_loc], FP8, tag=f"xb{kt}")
                nc.scalar.activation(xb[:], stg[:], SIGN, bias=sign_bias[:])
                xb_tiles.append(xb)

            def kxn_producer(nc_, md):
                return xb_tiles[md.k_tile_idx][:, :, ts(md.n_tile_idx, md.n_tile)]

            def kxm_producer(nc_, md):
                stg = wstage.tile([P, KT, md.m_tile], F32, tag="wstage")
                nc_.sync.dma_start(
                    stg[:],
                    wT_t[:, ts(md.k_tile_idx, KT), ts(md.m_tile_idx, md.m_tile)],
                )
                wb = kxm_pool.tile([P, KT, md.m_tile], FP8, tag="wb")
                nc_.scalar.activation(wb[:], stg[:], SIGN, bias=sign_bias[:])
                return wb

            def reducer(nc_, psum, sbuf, md):
                # psum: [128(o), FREE] ; bias is per-partition in this orientation.
                col = md.m_tile_idx * (md.m_tile // P) + md.m_subtile_idx
                nc_.vector.tensor_scalar(
                    sbuf,
                    psum,
                    bias_sb[:, col : col + 1],
                    None,
                    mybir.AluOpType.add,
                )

            composable_matmul_tile_kernel(
                tc=tc,
                kxm_shape=ShapeInfo(pdims=((P, d_in // P),), fdims=(d_out,)),
                kxn_shape=ShapeInfo(pdims=((P, d_in // P),), fdims=(m_loc,)),
                output_type=F32,
                kxm_producer=kxm_producer,
                kxn_producer=kxn_producer,
                mxn_consumer=dma_to_dram_mxn(outT[:]),
                mxn_subtile_reducer=reducer,
                MATMUL_FREE_DIM=512,
                MAX_TILE_SIZE=512,
                MAX_K_TILE_SIZE=P * KT,
                cache_tiles=True,
                temps_n_bufs=3,
                psum_n_bufs=2,
            )

    nc.compile()
    return nc


_NC_CACHE = None


def _get_nc():
    global _NC_CACHE
    if _NC_CACHE is None:
        _NC_CACHE = build()
    return _NC_CACHE


def _ensure_ntff_hook():
    """Register the axon NTFF profile hook if the image's antenv lacks it.

    bass_utils reads the hook via antenv.axon_hooks; this container's antenv has
    no axon_hooks module, but the slim boot package ships the ctypes equivalent.
    """
    try:
        from antenv.axon_hooks import get_axon_ntff_profile_hook  # noqa: F401

        return True
    except ImportError:
        pass
    try:
        import types

        import antenv
        from trn_agent_boot.trn_boot import _ntff_profile_via_ctypes

        hook = _ntff_profile_via_ctypes("/opt/axon/libaxon_pjrt.so")
        if hook is None:
            return False
        mod = types.ModuleType("antenv.axon_hooks")
        state = {"hook": hook}
        mod.set_axon_ntff_profile_hook = lambda h: state.update(hook=h)
        mod.get_axon_ntff_profile_hook = lambda: state["hook"]
        sys.modules["antenv.axon_hooks"] = mod
        antenv.axon_hooks = mod
        return True
    except Exception as e:  # profiling is best-effort; never block execution
        print(f"NTFF hook setup failed ({type(e).__name__}: {e}); no HW timing",
              file=sys.stderr)
        return False


def kernel(x, weight, bias):
    global LAST_EXEC_TIME_NS
    x = np.ascontiguousarray(np.asarray(x, dtype=np.float32))
    weight = np.asarray(weight, dtype=np.float32)
    bias = np.ascontiguousarray(np.asarray(bias, dtype=np.float32))
    wT = np.ascontiguousarray(weight.T)

    in_maps = []
    for c in range(NCORES):
        xT_c = np.ascontiguousarray(x[c * M_LOC : (c + 1) * M_LOC, :].T)
        in_maps.append({"xT": xT_c, "wT": wT, "bias": bias})

    nc = _get_nc()
    trace = os.environ.get("BINLIN_TRACE", "0") == "1"
    if trace:
        trace = _ensure_ntff_hook()
    core_ids = list(range(NCORES))
    if trace:
        try:
            res = run_bass_kernel_spmd(nc, in_maps, core_ids=core_ids, trace=True)
        except Exception as e:
            print(f"traced run failed ({type(e).__name__}: {e}); retry untraced",
                  file=sys.stderr)
            res = run_bass_kernel_spmd(nc, in_maps, core_ids=core_ids, trace=False)
    else:
        res = run_bass_kernel_spmd(nc, in_maps, core_ids=core_ids, trace=False)
    if res.exec_time_ns is not None:
        LAST_EXEC_TIME_NS = res.exec_time_ns

    outT = np.empty((D_OUT, N_TOK), dtype=np.float32)
    for c in range(NCORES):
        outT[:, c * M_LOC : (c + 1) * M_LOC] = res.results[c]["outT"]
    return np.ascontiguousarray(outT.T)


# revision 16
# speedup vs baseline: 1.1159x; 1.1159x over previous
"""BinarizeLinear Trainium2 kernel.

Computes y = binarize(x) @ binarize(W)^T + bias where binarize(t) = where(t>0, +1, -1),
x: [8192, 4096] f32, W: [4096, 4096] f32, bias: [4096] f32.

Strategy (8 NeuronCores, data parallel over tokens):
  - Each core gets 1024 tokens: xT shard [D_IN, 1024] (host-transposed layout so the
    contraction dim lands on SBUF partitions), the full W^T [D_IN, D_OUT], and bias.
  - On device: binarize via ScalarE Sign(t - 1e-20) (exact where(t>0,+1,-1) semantics,
    incl. t==0 -> -1) into fp8e4m3 (+/-1 exact in fp8; accumulation is fp32 PSUM so the
    whole matmul is exact). x kept resident in SBUF as fp8; W streamed fp32 -> fp8.
  - TensorE matmul in DoubleRow fp8 mode (2 MACs/cell/cycle), out^T[o, m] tiles in PSUM,
    bias added during the PSUM->SBUF drain on VectorE (bias is per-partition in this
    orientation), then DMA to DRAM as outT [D_OUT, 1024].
  - Host gathers the 8 outT shards and transposes back to [8192, 4096].
"""

import os
import sys

import numpy as np

sys.path.insert(0, "/opt/trn_rl_repo")

import concourse.bacc as bacc
import concourse.mybir as mybir
import concourse.tile as tile
from concourse.bass import ds, ts
from concourse.bass_utils import run_bass_kernel_spmd
from concourse.kernels.tile_matmul import (
    ShapeInfo,
    composable_matmul_tile_kernel,
)

N_TOK, D_IN, D_OUT = 8192, 4096, 4096
NCORES = 8
M_LOC = N_TOK // NCORES  # tokens per core
P = 128
KT = 4  # K subtiles per K tile (K_TILE = 512)
NEG_TINY = -1.0e-20  # Sign(t + NEG_TINY): t>0 -> +1, t<=0 -> -1

F32 = mybir.dt.float32
BF16 = mybir.dt.bfloat16
FP8 = mybir.dt.float8e4
SIGN = mybir.ActivationFunctionType.Sign
IDENT = mybir.ActivationFunctionType.Identity

LAST_EXEC_TIME_NS = None


def build(d_in=D_IN, d_out=D_OUT, m_loc=M_LOC):
    """Build the per-core Bass program (SPMD: all cores run the same NEFF)."""
    xbin = os.environ.get("BINLIN_XBIN", "dve")  # dve | act
    drain = os.environ.get("BINLIN_DRAIN", "mixed")  # mixed | dve
    out_eng = os.environ.get("BINLIN_OUT", "gpsimd")  # gpsimd | sync
    k_tiles = d_in // (P * KT)
    nc = bacc.Bacc("TRN2", target_bir_lowering=False, debug=False)

    xT = nc.dram_tensor("xT", (d_in, m_loc), F32, kind="ExternalInput")
    wT = nc.dram_tensor("wT", (d_in, d_out), F32, kind="ExternalInput")
    bias = nc.dram_tensor("bias", (d_out,), F32, kind="ExternalInput")
    outT = nc.dram_tensor("outT", (d_out, m_loc), F32, kind="ExternalOutput")

    with tile.TileContext(nc) as tc:
        with (
            tc.tile_pool(name="const", bufs=1) as const,
            tc.tile_pool(name="xstage", bufs=2) as xstage,
            tc.tile_pool(name="xb", bufs=1) as xbpool,
            tc.tile_pool(name="wstage", bufs=4) as wstage,
            tc.tile_pool(name="kxm", bufs=k_tiles + 1) as kxm_pool,
        ):
            bias_sb = const.tile([P, d_out // P], F32)
            nc.sync.dma_start(bias_sb[:], bias[:].rearrange("(o p) -> p o", p=P))

            # Per-partition bias for Sign(t + NEG_TINY) (float biases need a const AP)
            sign_bias = const.tile([P, 1], F32)
            nc.any.memset(sign_bias[:], NEG_TINY)

            xT_t = xT[:].rearrange("(po pi) m -> pi po m", pi=P)  # [128, d_in/128, m]
            wT_t = wT[:].rearrange("(po pi) o -> pi po o", pi=P)  # [128, d_in/128, o]

            # Load + binarize x once; keep resident in SBUF as fp8 (one tile per K_TILE
            # so matmul dependencies are per-K-tile, letting PE start before the full
            # load finishes). Engine routing matters: x DMAs ride the scalar HWDGE ring
            # (W stream owns the sync ring), and x binarize runs on VectorE so the
            # fill-gated x ops never queue ahead of W-binarize work in the ACT FIFO.
            # VectorE has no Sign activation, so binarize is two exact tensor_scalar
            # ops: u = (t > 0) in {0,1}, then 2u - 1 in {-1,+1}.
            xb_tiles = []
            for kt in range(k_tiles):
                stg = xstage.tile([P, KT, m_loc], F32, tag="xstage")
                nc.scalar.dma_start(stg[:], xT_t[:, ts(kt, KT), :])
                xb = xbpool.tile([P, KT, m_loc], FP8, tag=f"xb{kt}")
                if xbin == "dve":
                    xu = xstage.tile([P, KT, m_loc], BF16, tag="xu")
                    nc.vector.tensor_scalar(
                        xu[:], stg[:], 0.0, None, mybir.AluOpType.is_gt
                    )
                    nc.vector.tensor_scalar(
                        xb[:], xu[:], 2.0, -1.0,
                        mybir.AluOpType.mult, mybir.AluOpType.add,
                    )
                else:
                    nc.scalar.activation(xb[:], stg[:], SIGN, bias=sign_bias[:])
                xb_tiles.append(xb)

            def kxn_producer(nc_, md):
                return xb_tiles[md.k_tile_idx][:, :, ts(md.n_tile_idx, md.n_tile)]

            def kxm_producer(nc_, md):
                stg = wstage.tile([P, KT, md.m_tile], F32, tag="wstage")
                nc_.sync.dma_start(
                    stg[:],
                    wT_t[:, ts(md.k_tile_idx, KT), ts(md.m_tile_idx, md.m_tile)],
                )
                wb = kxm_pool.tile([P, KT, md.m_tile], FP8, tag="wb")
                nc_.scalar.activation(wb[:], stg[:], SIGN, bias=sign_bias[:])
                return wb

            def reducer(nc_, psum, sbuf, md):
                # psum: [128(o), FREE] ; bias is per-partition in this orientation.
                # Alternate drains between ACT and DVE by n-parity: the n1 drain is
                # what gates the next m-tile's W binarize in the ACT FIFO, so it goes
                # to DVE; n0 drains stay on ACT (closer to PSUM, bias is free).
                col = md.m_tile_idx * (md.m_tile // P) + md.m_subtile_idx
                if drain == "mixed" and md.n_tile_idx % 2 == 0:
                    nc_.scalar.activation(
                        sbuf, psum, IDENT, bias=bias_sb[:, col : col + 1]
                    )
                else:
                    nc_.vector.tensor_scalar(
                        sbuf,
                        psum,
                        bias_sb[:, col : col + 1],
                        None,
                        mybir.AluOpType.add,
                    )

            # Output stores ride the (otherwise idle) GpSimd SWDGE ring so they never
            # head-of-line-block the W loads on the sync ring.
            outT_t = outT[:].rearrange("(po pi) f -> pi po f", pi=P)

            def consumer(nc_, mxn_tile, md):
                eng = nc_.gpsimd if out_eng == "gpsimd" else nc_.sync
                eng.dma_start(
                    outT_t[
                        :,
                        ts(md.m_tile_idx, md.m_subtiles),
                        ds(md.n_tile_idx * md.n_tile, md.n_slice_size),
                    ],
                    mxn_tile[:, :, : md.n_slice_size],
                )

            composable_matmul_tile_kernel(
                tc=tc,
                kxm_shape=ShapeInfo(pdims=((P, d_in // P),), fdims=(d_out,)),
                kxn_shape=ShapeInfo(pdims=((P, d_in // P),), fdims=(m_loc,)),
                output_type=F32,
                kxm_producer=kxm_producer,
                kxn_producer=kxn_producer,
                mxn_consumer=consumer,
                mxn_subtile_reducer=reducer,
                MATMUL_FREE_DIM=512,
                MAX_TILE_SIZE=512,
                MAX_K_TILE_SIZE=P * KT,
                cache_tiles=True,
                temps_n_bufs=3,
                psum_n_bufs=2,
            )

    nc.compile()
    return nc


_NC_CACHE = None


def _get_nc():
    global _NC_CACHE
    if _NC_CACHE is None:
        _NC_CACHE = build()
    return _NC_CACHE


def _ensure_ntff_hook():
    """Register the axon NTFF profile hook if the image's antenv lacks it.

    bass_utils reads the hook via antenv.axon_hooks; this container's antenv has
    no axon_hooks module, but the slim boot package ships the ctypes equivalent.
    """
    try:
        from antenv.axon_hooks import get_axon_ntff_profile_hook  # noqa: F401

        return True
    except ImportError:
        pass
    try:
        import types

        import antenv
        from trn_agent_boot.trn_boot import _ntff_profile_via_ctypes

        hook = _ntff_profile_via_ctypes("/opt/axon/libaxon_pjrt.so")
        if hook is None:
            return False
        mod = types.ModuleType("antenv.axon_hooks")
        state = {"hook": hook}
        mod.set_axon_ntff_profile_hook = lambda h: state.update(hook=h)
        mod.get_axon_ntff_profile_hook = lambda: state["hook"]
        sys.modules["antenv.axon_hooks"] = mod
        antenv.axon_hooks = mod
        return True
    except Exception as e:  # profiling is best-effort; never block execution
        print(f"NTFF hook setup failed ({type(e).__name__}: {e}); no HW timing",
              file=sys.stderr)
        return False


def kernel(x, weight, bias):
    global LAST_EXEC_TIME_NS
    x = np.ascontiguousarray(np.asarray(x, dtype=np.float32))
    weight = np.asarray(weight, dtype=np.float32)
    bias = np.ascontiguousarray(np.asarray(bias, dtype=np.float32))
    wT = np.ascontiguousarray(weight.T)

    in_maps = []
    for c in range(NCORES):
        xT_c = np.ascontiguousarray(x[c * M_LOC : (c + 1) * M_LOC, :].T)
        in_maps.append({"xT": xT_c, "wT": wT, "bias": bias})

    nc = _get_nc()
    trace = os.environ.get("BINLIN_TRACE", "0") == "1"
    if trace:
        trace = _ensure_ntff_hook()
    core_ids = list(range(NCORES))
    if trace:
        try:
            res = run_bass_kernel_spmd(nc, in_maps, core_ids=core_ids, trace=True)
        except Exception as e:
            print(f"traced run failed ({type(e).__name__}: {e}); retry untraced",
                  file=sys.stderr)
            res = run_bass_kernel_spmd(nc, in_maps, core_ids=core_ids, trace=False)
    else:
        res = run_bass_kernel_spmd(nc, in_maps, core_ids=core_ids, trace=False)
    if res.exec_time_ns is not None:
        LAST_EXEC_TIME_NS = res.exec_time_ns

    outT = np.empty((D_OUT, N_TOK), dtype=np.float32)
    for c in range(NCORES):
        outT[:, c * M_LOC : (c + 1) * M_LOC] = res.results[c]["outT"]
    return np.ascontiguousarray(outT.T)
